# revision 3
# baseline (speedup 1.0000x reference)
"""Trainium2 Bass kernel for a 6-layer binary CNN (XNOR-net style).

Contract: kernel(**inputs) takes the FULL unsharded inputs (batch 128) and
returns the FULL output [128, 4, 4, 10] float32.

Strategy
--------
Pure data parallel: batch 128 -> 16 images on each of 8 NeuronCores; all
weights replicated. One SPMD Bass program, per-core input maps.

Per core:
  conv1 (3->128, fp32):  host-side im2col (K=27) packed 4-way into partition
      row-groups; 4-way row-tiled fp32 matmuls on the PE.
  conv2..conv6 (binary): sign(w) and sign(act) are exactly representable in
      fp8e4m3; products are +-1 and PSUM accumulates in fp32 -> the conv sums
      are EXACT integers.  3x3 SAME conv = 9 shifted matmuls accumulating in
      PSUM, reading from zero-haloed padded activation buffers in SBUF.
  relu/maxpool: relu commutes with max, so pool directly from PSUM with two
      tensor_tensor max ops, then fused (max(x,0)*scale) tensor_scalar, then
      ACT Sign (+bias) writes the fp8 binarized input of the next layer.
  dense+softmax: fp32 matmuls (h6 as stationary operand), bias via a K=1
      matmul against a ones vector, softmax with exp/accum on ACT.

All element-wise fp32 ops follow the reference's rounding sequence
(mul-round-add-round), so layers 2..6 are bit-exact vs the JAX reference;
the only inexactness is conv1 accumulation order and dense/softmax rounding.
"""

import numpy as np
import ml_dtypes

_F8 = ml_dtypes.float8_e4m3

B = 16        # images per core
N_CORES = 8

# (layer, Gi, Go, H, W, pool)
_LAYERS = [
    (2, 1, 1, 32, 32, True),
    (3, 1, 2, 16, 16, False),
    (4, 2, 2, 16, 16, True),
    (5, 2, 4, 8, 8, False),
    (6, 4, 4, 8, 8, True),
]
_WCOLS = {2: 1152, 3: 2304, 4: 4608, 5: 9216, 6: 18432}

_prog_cache = {}


def _build_program():
    """Build + compile the SPMD Bass program (once per process)."""
    if "nc" in _prog_cache:
        return _prog_cache["nc"]

    from contextlib import ExitStack

    import concourse.bacc as bacc
    import concourse.mybir as mybir
    import concourse.tile as tile

    dt = mybir.dt
    AL = mybir.AluOpType
    AF = mybir.ActivationFunctionType
    AX = mybir.AxisListType

    nc = bacc.Bacc("TRN2", target_bir_lowering=False, debug=False,
                   num_devices=N_CORES)

    f32 = dt.float32
    f8 = dt.float8e4

    d_xcol = nc.dram_tensor("xcol", [128, 4096], f32, kind="ExternalInput").ap()
    d_w1p = nc.dram_tensor("w1p", [128, 128], f32, kind="ExternalInput").ap()
    d_bnv = nc.dram_tensor("bnv", [128, 29], f32, kind="ExternalInput").ap()
    d_dwp = nc.dram_tensor("dwp", [128, 40], f32, kind="ExternalInput").ap()
    d_db = nc.dram_tensor("db", [1, 10], f32, kind="ExternalInput").ap()
    d_w = {l: nc.dram_tensor(f"wb{l}", [128, _WCOLS[l]], f8,
                             kind="ExternalInput").ap()
           for l, *_ in _LAYERS}
    d_out = nc.dram_tensor("out", [256, 10], f32, kind="ExternalOutput").ap()

    with tile.TileContext(nc) as tc, ExitStack() as ctx:
        consts = ctx.enter_context(tc.tile_pool(name="consts", bufs=1))
        psum_pool = ctx.enter_context(
            tc.tile_pool(name="cpsum", bufs=6, space="PSUM"))
        psum_d = ctx.enter_context(
            tc.tile_pool(name="dpsum", bufs=2, space="PSUM"))
        tmps = ctx.enter_context(tc.tile_pool(name="tmps", bufs=4))
        small = ctx.enter_context(tc.tile_pool(name="small", bufs=2))

        # ---- constant loads -------------------------------------------------
        xcol_sb = consts.tile([128, 4096], f32, tag="xcol")
        nc.sync.dma_start(xcol_sb[:], d_xcol)
        w1_sb = consts.tile([128, 128], f32, tag="w1p")
        nc.sync.dma_start(w1_sb[:], d_w1p)
        bn_sb = consts.tile([128, 29], f32, tag="bnv")
        nc.sync.dma_start(bn_sb[:], d_bnv)
        dwp_sb = consts.tile([128, 40], f32, tag="dwp")
        nc.sync.dma_start(dwp_sb[:], d_dwp)
        db_sb = consts.tile([1, 10], f32, tag="db")
        nc.sync.dma_start(db_sb[:], d_db)
        w_sb = {}
        for l, *_ in _LAYERS:
            w_sb[l] = consts.tile([128, _WCOLS[l]], f8, tag=f"wb{l}", name=f"wb{l}")
            nc.sync.dma_start(w_sb[l][:], d_w[l])
        ones_sb = consts.tile([1, 128], f32, tag="ones")
        nc.vector.memset(ones_sb[:], 1.0)

        # ---- padded (zero halo) binarized activation buffers, fp8 ----------
        def padded(name, hp, wp, n):
            bufs = []
            for i in range(n):
                t = consts.tile([128, B, hp, wp], f8, tag=f"{name}{i}", name=f"{name}{i}")
                nc.gpsimd.memset(t[:], 0.0)
                bufs.append(t)
            return bufs

        a_in = {2: padded("a2", 34, 34, 1),
                3: padded("a3", 18, 18, 1),
                4: padded("a4", 18, 18, 2),
                5: padded("a5", 10, 10, 2),
                6: padded("a6", 10, 10, 4)}
        h6 = [consts.tile([128, B, 4, 4], f32, tag=f"h6{i}", name=f"h6{i}") for i in range(4)]

        # bn vector columns in bn_sb
        bn_cols = {1: (1, 2), 2: (3, 4), 3: (5, 7), 4: (9, 11),
                   5: (13, 17), 6: (21, 25)}
        b1_ap = bn_sb[:, 0:1]

        # ---- layer 1: fp32 conv via 4-way row-tiled K=27 matmuls ------------
        s1_ap = bn_sb[:, 1:2]
        bb1_ap = bn_sb[:, 2:3]
        for g in range(4):
            for ns in range(8):
                b = 4 * g + ns // 2
                y0 = (ns % 2) * 16
                pt = psum_pool.tile([128, 16, 32], f32, tag="cps", name="cps")
                lhsT = w1_sb[32 * g:32 * g + 27, :]
                rhs = xcol_sb[32 * g:32 * g + 27, 512 * ns:512 * (ns + 1)]
                nc.tensor.matmul(pt[:, :, :], lhsT, rhs, start=True, stop=True,
                                 tile_position=(32 * g, 0))
                r = tmps.tile([128, 16, 32], f32, tag="rl1", name="rl1")
                # relu(conv + b1)
                nc.vector.tensor_scalar(r[:], pt[:, :, :], b1_ap, 0.0,
                                        AL.add, AL.max)
                # sign(s1 * r + bb1) -> fp8 into padded L2 input
                dest = a_in[2][0][:, b, 1 + y0:17 + y0, 1:33]
                nc.scalar.activation(dest, r[:], AF.Sign,
                                     bias=bb1_ap, scale=s1_ap)

        # ---- binary conv layers --------------------------------------------
        for (l, Gi, Go, H, W, pool) in _LAYERS:
            HW = H * W
            if HW >= 512:
                nb = 1
                rows = 512 // W               # rows of one image per chunk
                chunks_per_img = H // rows
                nchunks = B * chunks_per_img
            else:
                nb = 512 // HW
                rows = nb * H
                nchunks = B // nb
            sc0, bc0 = bn_cols[l]
            acts = a_in[l]
            Hp2, Wp2 = H // 2, W // 2
            for go in range(Go):
                s_ap = bn_sb[:, sc0 + go:sc0 + go + 1]
                b_ap = bn_sb[:, bc0 + go:bc0 + go + 1]
                for c in range(nchunks):
                    if nb == 1:
                        b0 = c // chunks_per_img
                        y0 = (c % chunks_per_img) * rows
                    else:
                        b0 = c * nb
                        y0 = 0
                    pt = psum_pool.tile([128, rows, W], f32, tag="cps", name="cps")
                    n_mm = Gi * 9
                    i_mm = 0
                    for gi in range(Gi):
                        for k in range(9):
                            dy, dx = k // 3, k % 3
                            col = ((gi * Go + go) * 9 + k) * 128
                            lhsT = w_sb[l][:, col:col + 128]
                            if nb == 1:
                                rhs = acts[gi][:, b0, y0 + dy:y0 + dy + rows,
                                               dx:dx + W]
                            else:
                                rhs = acts[gi][:, b0:b0 + nb, dy:dy + H,
                                               dx:dx + W]
                            nc.tensor.matmul(pt[:, :, :], lhsT, rhs,
                                             start=(i_mm == 0),
                                             stop=(i_mm == n_mm - 1))
                            i_mm += 1
                    # relu+scale straight out of PSUM; s>0 commutes with
                    # max, so pooling after scaling is bit-identical
                    ts = tmps.tile([128, rows, W], f32, tag="ts", name="ts")
                    nc.vector.tensor_scalar(ts[:], pt[:, :, :], 0.0, s_ap,
                                            AL.max, AL.mult)
                    if pool:
                        vx = ts[:].rearrange(
                            "p r (a two) -> p r a two", two=2)
                        tx = tmps.tile([128, rows, W // 2], f32, tag="tx", name="tx")
                        nc.vector.tensor_tensor(
                            tx[:], vx[:, :, :, 0], vx[:, :, :, 1], op=AL.max)
                        vy = tx[:].rearrange(
                            "p (a two) x -> p a two x", two=2)
                        tp = tmps.tile([128, rows // 2, W // 2], f32, tag="tp", name="tp")
                        nc.vector.tensor_tensor(
                            tp[:], vy[:, :, 0, :], vy[:, :, 1, :], op=AL.max)
                        src = tp
                    else:
                        src = ts
                    if l < 6:
                        # sign(x + bias) -> fp8 into next layer's padded input
                        nxt = a_in[l + 1][go]
                        if pool:
                            if nb == 1:   # L2 -> L3 input
                                dest = nxt[:, b0, 1 + y0 // 2:
                                           1 + y0 // 2 + rows // 2, 1:1 + Wp2]
                                src_ap = src[:]
                            else:
                                dest = nxt[:, b0:b0 + nb, 1:1 + Hp2, 1:1 + Wp2]
                                src_ap = src[:].rearrange(
                                    "p (b y) x -> p b y x", b=nb)
                        else:
                            dest = nxt[:, b0:b0 + nb, 1:1 + H, 1:1 + W]
                            src_ap = src[:].rearrange(
                                "p (b y) x -> p b y x", b=nb)
                        nc.scalar.activation(dest, src_ap, AF.Sign, bias=b_ap)
                    else:
                        # L6: h6 = x + bias (fp32, feeds dense)
                        dest = h6[go][:, b0:b0 + nb, :, :]
                        src_ap = src[:].rearrange("p (b y) x -> p b y x", b=nb)
                        nc.scalar.activation(dest, src_ap, AF.Identity,
                                             bias=b_ap)

        # ---- dense + softmax ------------------------------------------------
        for p in range(2):
            ptd = psum_d.tile([128, 10], f32, tag="dps", name="dps")
            for gi in range(4):
                lhsT = h6[gi][:, 8 * p:8 * p + 8, :, :]
                rhs = dwp_sb[:, gi * 10:(gi + 1) * 10]
                nc.tensor.matmul(ptd[:, :], lhsT, rhs,
                                 start=(gi == 0), stop=False)
            nc.tensor.matmul(ptd[:, :], ones_sb[0:1, :], db_sb[0:1, :],
                             start=False, stop=True)
            mx = small.tile([128, 1], f32, tag="mx", name="mx")
            nc.vector.tensor_reduce(mx[:], ptd[:, :], axis=AX.X, op=AL.max,
                                    negate=True)
            e = small.tile([128, 10], f32, tag="e", name="e")
            ssum = small.tile([128, 1], f32, tag="ssum", name="ssum")
            nc.scalar.activation(e[:], ptd[:, :], AF.Exp, bias=mx[:],
                                 scale=1.0, accum_out=ssum[:])
            rcp = small.tile([128, 1], f32, tag="rcp", name="rcp")
            nc.vector.reciprocal(rcp[:], ssum[:])
            o = small.tile([128, 10], f32, tag="o", name="o")
            nc.vector.tensor_scalar(o[:], e[:], rcp[:], None, AL.mult)
            nc.sync.dma_start(d_out[128 * p:128 * (p + 1), :], o[:])

    nc.compile()
    _prog_cache["nc"] = nc
    return nc


# --------------------------------------------------------------------------
# host-side input packing
# --------------------------------------------------------------------------

def _pack_shared(inputs):
    w1 = np.asarray(inputs["w1"], np.float32)
    w1flat = w1.reshape(27, 128)          # row r = (ky*3+kx)*3 + ci
    w1p = np.zeros((128, 128), np.float32)
    for g in range(4):
        w1p[32 * g:32 * g + 27, :] = w1flat

    bnv = np.zeros((128, 29), np.float32)
    bnv[:, 0] = np.asarray(inputs["b1"], np.float32)
    bnv[:, 1] = np.asarray(inputs["bn1_scale"], np.float32)
    bnv[:, 2] = np.asarray(inputs["bn1_bias"], np.float32)
    bn_cols = {2: (3, 4), 3: (5, 7), 4: (9, 11), 5: (13, 17), 6: (21, 25)}
    for l, (sc, bc) in bn_cols.items():
        s = np.asarray(inputs[f"bn{l}_scale"], np.float32)
        b = np.asarray(inputs[f"bn{l}_bias"], np.float32)
        g = s.size // 128
        bnv[:, sc:sc + g] = s.reshape(g, 128).T
        bnv[:, bc:bc + g] = b.reshape(g, 128).T

    wbs = {}
    for (l, Gi, Go, _, _, _) in _LAYERS:
        w = np.asarray(inputs[f"w{l}"], np.float32)
        ws = np.sign(w).astype(_F8)       # (3,3,Cin,Cout)
        blob = np.empty((128, _WCOLS[l]), _F8)
        for gi in range(Gi):
            for go in range(Go):
                for k in range(9):
                    col = ((gi * Go + go) * 9 + k) * 128
                    blob[:, col:col + 128] = ws[k // 3, k % 3,
                                                gi * 128:(gi + 1) * 128,
                                                go * 128:(go + 1) * 128]
        wbs[l] = blob

    dw = np.asarray(inputs["dense_w"], np.float32)
    dwp = dw.reshape(4, 128, 10).transpose(1, 0, 2).reshape(128, 40).copy()
    db = np.asarray(inputs["dense_b"], np.float32).reshape(1, 10).copy()
    return w1p, bnv, wbs, dwp, db


def _pack_xcol(x16):
    """[16,32,32,3] f32 -> [128,4096] 4-way row-group packed im2col."""
    xp = np.zeros((B, 34, 34, 3), np.float32)
    xp[:, 1:33, 1:33, :] = x16
    cols = np.empty((27, B, 32, 32), np.float32)
    for ky in range(3):
        for kx in range(3):
            for ci in range(3):
                r = (ky * 3 + kx) * 3 + ci
                cols[r] = xp[:, ky:ky + 32, kx:kx + 32, ci]
    cols = cols.reshape(27, B * 1024)
    xcol = np.zeros((128, 4096), np.float32)
    for g in range(4):
        xcol[32 * g:32 * g + 27, :] = cols[:, 4096 * g:4096 * (g + 1)]
    return xcol


def _make_in_maps(inputs):
    w1p, bnv, wbs, dwp, db = _pack_shared(inputs)
    x = np.asarray(inputs["x"], np.float32)
    in_maps = []
    for c in range(N_CORES):
        m = {"xcol": _pack_xcol(x[B * c:B * (c + 1)]),
             "w1p": w1p, "bnv": bnv, "dwp": dwp, "db": db}
        for l in wbs:
            m[f"wb{l}"] = wbs[l]
        in_maps.append(m)
    return in_maps


def _run(inputs, trace=False):
    """Returns (output [128,4,4,10] f32, BassKernelResults)."""
    nc = _build_program()
    from concourse.bass_utils import run_bass_kernel_spmd
    in_maps = _make_in_maps(inputs)
    res = run_bass_kernel_spmd(nc, in_maps, list(range(N_CORES)), trace=trace)
    outs = [res.results[c]["out"].reshape(B, 4, 4, 10)
            for c in range(N_CORES)]
    return np.concatenate(outs, axis=0), res


def kernel(**inputs):
    out, _ = _run(inputs)
    return out


# revision 5
# speedup vs baseline: 1.1750x; 1.1750x over previous
"""Trainium2 Bass kernel for a 6-layer binary CNN (XNOR-net style).

Contract: kernel(**inputs) takes the FULL unsharded inputs (batch 128) and
returns the FULL output [128, 4, 4, 10] float32.

Strategy
--------
Pure data parallel: batch 128 -> 16 images on each of 8 NeuronCores; all
weights replicated. One SPMD Bass program, per-core input maps.

Per core:
  conv1 (3->128, fp32):  host-side im2col (K=27) packed 4-way into partition
      row-groups; 4-way row-tiled fp32 matmuls on the PE.
  conv2..conv6 (binary): sign(w) and sign(act) are exactly representable in
      fp8e4m3; products are +-1 and PSUM accumulates in fp32 -> the conv sums
      are EXACT integers.  3x3 SAME conv = 9 shifted matmuls accumulating in
      PSUM, reading from zero-haloed padded activation buffers in SBUF.
  conv4..conv6 additionally run in fp8 DoubleRow mode (2 MACs/cell/cycle):
      activations for a pair of 128-channel groups live in one flat
      [128, 2, S] buffer (padded images stored contiguously incl. halos,
      plus guard columns), so the moving operand is a contiguous run
      [K, 2, N] over full padded images; conv outputs at halo positions are
      garbage and are simply never read by the strided post-processing.
  relu/maxpool: relu and (positive) BN scale commute with max, so pool after
      the fused (max(x,0)*scale) tensor_scalar; then ACT Sign (+bias) writes
      the fp8 binarized input of the next layer.
  dense+softmax: fp32 matmuls (h6 as stationary operand), bias via a K=1
      matmul against a ones vector, softmax with exp/accum on ACT.

All element-wise fp32 ops follow the reference's rounding sequence
(mul-round-add-round), so layers 2..6 are bit-exact vs the JAX reference;
the only inexactness is conv1 accumulation order and dense/softmax rounding.
"""

import numpy as np
import ml_dtypes

_F8 = ml_dtypes.float8_e4m3

B = 16        # images per core
N_CORES = 8

# (layer, Gi, Go, H, W, pool, doublerow)
_LAYERS = [
    (2, 1, 1, 32, 32, True, False),
    (3, 1, 2, 16, 16, False, False),
    (4, 2, 2, 16, 16, True, True),
    (5, 2, 4, 8, 8, False, True),
    (6, 4, 4, 8, 8, True, True),
]
_WCOLS = {2: 1152, 3: 2304, 4: 4608, 5: 9216, 6: 18432}


def _flat_geom(H, W):
    """Geometry of the flat padded pair-buffers for DoubleRow layers."""
    hp, wp = H + 2, W + 2
    img = hp * wp
    g = ((wp + 1 + 15) // 16) * 16      # guard >= wp+1, multiple of 16
    s = B * img + 2 * g
    assert s % 16 == 0
    return hp, wp, img, g, s


_prog_cache = {}


def _build_program():
    """Build + compile the SPMD Bass program (once per process)."""
    if "nc" in _prog_cache:
        return _prog_cache["nc"]

    from contextlib import ExitStack

    import concourse.bacc as bacc
    import concourse.mybir as mybir
    import concourse.tile as tile

    dt = mybir.dt
    AL = mybir.AluOpType
    AF = mybir.ActivationFunctionType
    AX = mybir.AxisListType
    DR = mybir.MatmulPerfMode.DoubleRow

    nc = bacc.Bacc("TRN2", target_bir_lowering=False, debug=False,
                   num_devices=N_CORES)

    f32 = dt.float32
    f8 = dt.float8e4

    d_xcol = nc.dram_tensor("xcol", [128, 4096], f32, kind="ExternalInput").ap()
    d_w1p = nc.dram_tensor("w1p", [128, 128], f32, kind="ExternalInput").ap()
    d_bnv = nc.dram_tensor("bnv", [128, 29], f32, kind="ExternalInput").ap()
    d_dwp = nc.dram_tensor("dwp", [128, 40], f32, kind="ExternalInput").ap()
    d_db = nc.dram_tensor("db", [1, 10], f32, kind="ExternalInput").ap()
    d_w = {l: nc.dram_tensor(f"wb{l}", [128, _WCOLS[l]], f8,
                             kind="ExternalInput").ap()
           for l, *_ in _LAYERS}
    d_out = nc.dram_tensor("out", [256, 10], f32, kind="ExternalOutput").ap()

    g4 = _flat_geom(16, 16)   # a4 geometry (L4 input 16x16)
    g5 = _flat_geom(8, 8)     # a5
    g6 = _flat_geom(8, 8)     # a6

    with tile.TileContext(nc) as tc, ExitStack() as ctx:
        consts = ctx.enter_context(tc.tile_pool(name="consts", bufs=1))
        psum_pool = ctx.enter_context(
            tc.tile_pool(name="cpsum", bufs=6, space="PSUM"))
        psum_d = ctx.enter_context(
            tc.tile_pool(name="dpsum", bufs=2, space="PSUM"))
        tmps = ctx.enter_context(tc.tile_pool(name="tmps", bufs=4))
        small = ctx.enter_context(tc.tile_pool(name="small", bufs=2))

        # ---- constant loads -------------------------------------------------
        xcol_sb = consts.tile([128, 4096], f32, tag="xcol")
        nc.sync.dma_start(xcol_sb[:], d_xcol)
        w1_sb = consts.tile([128, 128], f32, tag="w1p")
        nc.sync.dma_start(w1_sb[:], d_w1p)
        bn_sb = consts.tile([128, 29], f32, tag="bnv")
        nc.sync.dma_start(bn_sb[:], d_bnv)
        dwp_sb = consts.tile([128, 40], f32, tag="dwp")
        nc.sync.dma_start(dwp_sb[:], d_dwp)
        db_sb = consts.tile([1, 10], f32, tag="db")
        nc.sync.dma_start(db_sb[:], d_db)
        w_sb = {}
        for l, *_ in _LAYERS:
            w_sb[l] = consts.tile([128, _WCOLS[l]], f8, tag=f"wb{l}",
                                  name=f"wb{l}")
            nc.sync.dma_start(w_sb[l][:], d_w[l])
        ones_sb = consts.tile([1, 128], f32, tag="ones")
        nc.vector.memset(ones_sb[:], 1.0)

        # ---- activation buffers --------------------------------------------
        a2 = consts.tile([128, B, 34, 34], f8, tag="a2", name="a2")
        nc.gpsimd.memset(a2[:], 0.0)
        a3 = consts.tile([128, B, 18, 18], f8, tag="a3", name="a3")
        nc.gpsimd.memset(a3[:], 0.0)
        a4 = consts.tile([128, 2, g4[4]], f8, tag="a4", name="a4")
        nc.gpsimd.memset(a4[:], 0.0)
        a5 = consts.tile([128, 2, g5[4]], f8, tag="a5", name="a5")
        nc.gpsimd.memset(a5[:], 0.0)
        a6 = [consts.tile([128, 2, g6[4]], f8, tag=f"a6{i}", name=f"a6{i}")
              for i in range(2)]
        for t in a6:
            nc.gpsimd.memset(t[:], 0.0)
        h6 = [consts.tile([128, B, 4, 4], f32, tag=f"h6{i}", name=f"h6{i}")
              for i in range(4)]

        def flat_view(t, geom):
            """[128,2,S] -> [128, 2, B, Hp, Wp] view of the guarded region."""
            hp, wp, img, g, s = geom
            return t[:, :, g:g + B * img].rearrange(
                "p j (b h w) -> p j b h w", b=B, h=hp, w=wp)

        a4v = flat_view(a4, g4)
        a5v = flat_view(a5, g5)
        a6v = [flat_view(t, g6) for t in a6]

        bn_cols = {1: (1, 2), 2: (3, 4), 3: (5, 7), 4: (9, 11),
                   5: (13, 17), 6: (21, 25)}
        b1_ap = bn_sb[:, 0:1]

        # ---- layer 1: fp32 conv via 4-way row-tiled K=27 matmuls ------------
        s1_ap = bn_sb[:, 1:2]
        bb1_ap = bn_sb[:, 2:3]
        for g in range(4):
            for ns in range(8):
                b = 4 * g + ns // 2
                y0 = (ns % 2) * 16
                pt = psum_pool.tile([128, 16, 32], f32, tag="cps", name="cps")
                lhsT = w1_sb[32 * g:32 * g + 27, :]
                rhs = xcol_sb[32 * g:32 * g + 27, 512 * ns:512 * (ns + 1)]
                nc.tensor.matmul(pt[:, :, :], lhsT, rhs, start=True, stop=True,
                                 tile_position=(32 * g, 0))
                r = tmps.tile([128, 16, 32], f32, tag="rl1", name="rl1")
                nc.vector.tensor_scalar(r[:], pt[:, :, :], b1_ap, 0.0,
                                        AL.add, AL.max)
                dest = a2[:, b, 1 + y0:17 + y0, 1:33]
                nc.scalar.activation(dest, r[:], AF.Sign,
                                     bias=bb1_ap, scale=s1_ap)

        def write_next(l, go, b0, nb, src_ap, H2, W2, y0=0):
            """Write binarized/affine output into layer l+1's input buffer."""
            b_ap = bn_sb[:, bn_cols[l][1] + go:bn_cols[l][1] + go + 1]
            if l == 2:
                dest = a3[:, b0, 1 + y0 // 2:1 + y0 // 2 + H2, 1:1 + W2]
                nc.scalar.activation(dest, src_ap, AF.Sign, bias=b_ap)
            elif l == 3:
                dest = a4v[:, go, b0:b0 + nb, 1:1 + H2, 1:1 + W2]
                nc.scalar.activation(dest, src_ap, AF.Sign, bias=b_ap)
            elif l == 4:
                dest = a5v[:, go, b0:b0 + nb, 1:1 + H2, 1:1 + W2]
                nc.scalar.activation(dest, src_ap, AF.Sign, bias=b_ap)
            elif l == 5:
                dest = a6v[go // 2][:, go % 2, b0:b0 + nb, 1:1 + H2, 1:1 + W2]
                nc.scalar.activation(dest, src_ap, AF.Sign, bias=b_ap)
            else:
                dest = h6[go][:, b0:b0 + nb, :, :]
                nc.scalar.activation(dest, src_ap, AF.Identity, bias=b_ap)

        # ---- binary conv layers L2/L3 (classic shifted-AP path) -------------
        for (l, Gi, Go, H, W, pool, dr) in _LAYERS:
            if dr:
                continue
            sc0, _ = bn_cols[l]
            acts = {2: [a2], 3: [a3]}[l]
            if H * W >= 512:
                nb, rows = 1, 512 // W
                chunks_per_img = H // rows
                nchunks = B * chunks_per_img
            else:
                nb = 512 // (H * W)
                rows = nb * H
                nchunks = B // nb
            for go in range(Go):
                s_ap = bn_sb[:, sc0 + go:sc0 + go + 1]
                for c in range(nchunks):
                    if nb == 1:
                        b0, y0 = c // chunks_per_img, (c % chunks_per_img) * rows
                    else:
                        b0, y0 = c * nb, 0
                    pt = psum_pool.tile([128, rows, W], f32, tag="cps",
                                        name="cps")
                    i_mm, n_mm = 0, Gi * 9
                    for gi in range(Gi):
                        for k in range(9):
                            dy, dx = k // 3, k % 3
                            col = ((gi * Go + go) * 9 + k) * 128
                            lhsT = w_sb[l][:, col:col + 128]
                            if nb == 1:
                                rhs = acts[gi][:, b0, y0 + dy:y0 + dy + rows,
                                               dx:dx + W]
                            else:
                                rhs = acts[gi][:, b0:b0 + nb, dy:dy + H,
                                               dx:dx + W]
                            nc.tensor.matmul(pt[:, :, :], lhsT, rhs,
                                             start=(i_mm == 0),
                                             stop=(i_mm == n_mm - 1))
                            i_mm += 1
                    # relu+scale from PSUM; s>0 commutes with max
                    ts = tmps.tile([128, rows, W], f32, tag="ts", name="ts")
                    nc.vector.tensor_scalar(ts[:], pt[:, :, :], 0.0, s_ap,
                                            AL.max, AL.mult)
                    if pool:
                        vx = ts[:].rearrange("p r (a two) -> p r a two", two=2)
                        tx = tmps.tile([128, rows, W // 2], f32, tag="tx",
                                       name="tx")
                        nc.vector.tensor_tensor(tx[:], vx[:, :, :, 0],
                                                vx[:, :, :, 1], op=AL.max)
                        vy = tx[:].rearrange("p (a two) x -> p a two x", two=2)
                        tp = tmps.tile([128, rows // 2, W // 2], f32, tag="tp",
                                       name="tp")
                        nc.vector.tensor_tensor(tp[:], vy[:, :, 0, :],
                                                vy[:, :, 1, :], op=AL.max)
                        if nb == 1:
                            write_next(l, go, b0, 1, tp[:], rows // 2, W // 2,
                                       y0=y0)
                        else:
                            write_next(l, go, b0, nb,
                                       tp[:].rearrange("p (b y) x -> p b y x",
                                                       b=nb),
                                       H // 2, W // 2)
                    else:
                        write_next(l, go, b0, nb,
                                   ts[:].rearrange("p (b y) x -> p b y x",
                                                   b=nb), H, W)

        # ---- binary conv layers L4/L5/L6 (fp8 DoubleRow, flat runs) ---------
        for (l, Gi, Go, H, W, pool, dr) in _LAYERS:
            if not dr:
                continue
            sc0, _ = bn_cols[l]
            geom = {4: g4, 5: g5, 6: g6}[l]
            hp, wp, img, gd, s = geom
            srcs = {4: [a4], 5: [a5], 6: a6}[l]
            npairs = Gi // 2
            nb = 1                            # images per chunk (divisor of B)
            while nb * 2 <= B and nb * 2 * img <= 512:
                nb *= 2
            N = nb * img
            nchunks = B // nb
            for go in range(Go):
                s_ap = bn_sb[:, sc0 + go:sc0 + go + 1]
                for c in range(nchunks):
                    b0 = c * nb
                    pt = psum_pool.tile([128, N], f32, tag="cps", name="cps")
                    i_mm, n_mm = 0, npairs * 9
                    for pr in range(npairs):
                        for k in range(9):
                            dy, dx = k // 3, k % 3
                            base = ((pr * Go + go) * 9 + k) * 256
                            lhsT = w_sb[l][:, base:base + 256].rearrange(
                                "p (j c) -> p j c", j=2)
                            off = gd + b0 * img + (dy - 1) * wp + (dx - 1)
                            rhs = srcs[pr][:, :, off:off + N]
                            nc.tensor.matmul(pt[:, :], lhsT, rhs,
                                             start=(i_mm == 0),
                                             stop=(i_mm == n_mm - 1),
                                             perf_mode=DR)
                            i_mm += 1
                    # interior view of the padded-grid conv output
                    ptv = pt[:].rearrange("p (b h w) -> p b h w",
                                          b=nb, h=hp, w=wp)
                    inter = ptv[:, :, 1:1 + H, 1:1 + W]
                    ts = tmps.tile([128, nb, H, W], f32, tag="ts", name="ts")
                    nc.vector.tensor_scalar(ts[:], inter, 0.0, s_ap,
                                            AL.max, AL.mult)
                    if pool:
                        vx = ts[:].rearrange("p b h (x two) -> p b h x two",
                                             two=2)
                        tx = tmps.tile([128, nb, H, W // 2], f32, tag="tx",
                                       name="tx")
                        nc.vector.tensor_tensor(tx[:], vx[:, :, :, :, 0],
                                                vx[:, :, :, :, 1], op=AL.max)
                        vy = tx[:].rearrange("p b (y two) x -> p b y two x",
                                             two=2)
                        tp = tmps.tile([128, nb, H // 2, W // 2], f32,
                                       tag="tp", name="tp")
                        nc.vector.tensor_tensor(tp[:], vy[:, :, :, 0, :],
                                                vy[:, :, :, 1, :], op=AL.max)
                        write_next(l, go, b0, nb, tp[:], H // 2, W // 2)
                    else:
                        write_next(l, go, b0, nb, ts[:], H, W)

        # ---- dense + softmax ------------------------------------------------
        for p in range(2):
            ptd = psum_d.tile([128, 10], f32, tag="dps", name="dps")
            for gi in range(4):
                lhsT = h6[gi][:, 8 * p:8 * p + 8, :, :]
                rhs = dwp_sb[:, gi * 10:(gi + 1) * 10]
                nc.tensor.matmul(ptd[:, :], lhsT, rhs,
                                 start=(gi == 0), stop=False)
            nc.tensor.matmul(ptd[:, :], ones_sb[0:1, :], db_sb[0:1, :],
                             start=False, stop=True)
            mx = small.tile([128, 1], f32, tag="mx", name="mx")
            nc.vector.tensor_reduce(mx[:], ptd[:, :], axis=AX.X, op=AL.max,
                                    negate=True)
            e = small.tile([128, 10], f32, tag="e", name="e")
            ssum = small.tile([128, 1], f32, tag="ssum", name="ssum")
            nc.scalar.activation(e[:], ptd[:, :], AF.Exp, bias=mx[:],
                                 scale=1.0, accum_out=ssum[:])
            rcp = small.tile([128, 1], f32, tag="rcp", name="rcp")
            nc.vector.reciprocal(rcp[:], ssum[:])
            o = small.tile([128, 10], f32, tag="o", name="o")
            nc.vector.tensor_scalar(o[:], e[:], rcp[:], None, AL.mult)
            nc.sync.dma_start(d_out[128 * p:128 * (p + 1), :], o[:])

    nc.compile()
    _prog_cache["nc"] = nc
    return nc


# --------------------------------------------------------------------------
# host-side input packing
# --------------------------------------------------------------------------

def _pack_shared(inputs):
    w1 = np.asarray(inputs["w1"], np.float32)
    w1flat = w1.reshape(27, 128)          # row r = (ky*3+kx)*3 + ci
    w1p = np.zeros((128, 128), np.float32)
    for g in range(4):
        w1p[32 * g:32 * g + 27, :] = w1flat

    bnv = np.zeros((128, 29), np.float32)
    bnv[:, 0] = np.asarray(inputs["b1"], np.float32)
    bnv[:, 1] = np.asarray(inputs["bn1_scale"], np.float32)
    bnv[:, 2] = np.asarray(inputs["bn1_bias"], np.float32)
    bn_cols = {2: (3, 4), 3: (5, 7), 4: (9, 11), 5: (13, 17), 6: (21, 25)}
    for l, (sc, bc) in bn_cols.items():
        s = np.asarray(inputs[f"bn{l}_scale"], np.float32)
        b = np.asarray(inputs[f"bn{l}_bias"], np.float32)
        g = s.size // 128
        bnv[:, sc:sc + g] = s.reshape(g, 128).T
        bnv[:, bc:bc + g] = b.reshape(g, 128).T

    wbs = {}
    for (l, Gi, Go, _, _, _, dr) in _LAYERS:
        w = np.asarray(inputs[f"w{l}"], np.float32)
        ws = np.sign(w).astype(_F8)       # (3,3,Cin,Cout)
        blob = np.empty((128, _WCOLS[l]), _F8)
        if not dr:
            for gi in range(Gi):
                for go in range(Go):
                    for k in range(9):
                        col = ((gi * Go + go) * 9 + k) * 128
                        blob[:, col:col + 128] = ws[k // 3, k % 3,
                                                    gi * 128:(gi + 1) * 128,
                                                    go * 128:(go + 1) * 128]
        else:
            for pr in range(Gi // 2):
                for go in range(Go):
                    for k in range(9):
                        base = ((pr * Go + go) * 9 + k) * 256
                        for j in range(2):
                            ci0 = (2 * pr + j) * 128
                            blob[:, base + j * 128:base + (j + 1) * 128] = \
                                ws[k // 3, k % 3, ci0:ci0 + 128,
                                   go * 128:(go + 1) * 128]
        wbs[l] = blob

    dw = np.asarray(inputs["dense_w"], np.float32)
    dwp = dw.reshape(4, 128, 10).transpose(1, 0, 2).reshape(128, 40).copy()
    db = np.asarray(inputs["dense_b"], np.float32).reshape(1, 10).copy()
    return w1p, bnv, wbs, dwp, db


def _pack_xcol(x16):
    """[16,32,32,3] f32 -> [128,4096] 4-way row-group packed im2col."""
    xp = np.zeros((B, 34, 34, 3), np.float32)
    xp[:, 1:33, 1:33, :] = x16
    cols = np.empty((27, B, 32, 32), np.float32)
    for ky in range(3):
        for kx in range(3):
            for ci in range(3):
                r = (ky * 3 + kx) * 3 + ci
                cols[r] = xp[:, ky:ky + 32, kx:kx + 32, ci]
    cols = cols.reshape(27, B * 1024)
    xcol = np.zeros((128, 4096), np.float32)
    for g in range(4):
        xcol[32 * g:32 * g + 27, :] = cols[:, 4096 * g:4096 * (g + 1)]
    return xcol


def _make_in_maps(inputs):
    w1p, bnv, wbs, dwp, db = _pack_shared(inputs)
    x = np.asarray(inputs["x"], np.float32)
    in_maps = []
    for c in range(N_CORES):
        m = {"xcol": _pack_xcol(x[B * c:B * (c + 1)]),
             "w1p": w1p, "bnv": bnv, "dwp": dwp, "db": db}
        for l in wbs:
            m[f"wb{l}"] = wbs[l]
        in_maps.append(m)
    return in_maps


def _run(inputs, trace=False):
    """Returns (output [128,4,4,10] f32, BassKernelResults)."""
    nc = _build_program()
    from concourse.bass_utils import run_bass_kernel_spmd
    in_maps = _make_in_maps(inputs)
    res = run_bass_kernel_spmd(nc, in_maps, list(range(N_CORES)), trace=trace)
    outs = [res.results[c]["out"].reshape(B, 4, 4, 10)
            for c in range(N_CORES)]
    return np.concatenate(outs, axis=0), res


def kernel(**inputs):
    out, _ = _run(inputs)
    return out


# revision 6
# speedup vs baseline: 1.2632x; 1.0750x over previous
"""Trainium2 Bass kernel for a 6-layer binary CNN (XNOR-net style).

Contract: kernel(**inputs) takes the FULL unsharded inputs (batch 128) and
returns the FULL output [128, 4, 4, 10] float32.

Strategy
--------
Pure data parallel: batch 128 -> 16 images on each of 8 NeuronCores; all
weights replicated. One SPMD Bass program, per-core input maps.

Per core:
  conv1 (3->128, fp32):  host-side im2col (K=27) packed 4-way into partition
      row-groups; 4-way row-tiled fp32 matmuls on the PE.
  conv2..conv6 (binary): sign(w) and sign(act) are exactly representable in
      fp8e4m3; products are +-1 and PSUM accumulates in fp32 -> the conv sums
      are EXACT integers.  3x3 SAME conv = 9 shifted matmuls accumulating in
      PSUM, reading from zero-haloed padded activation buffers in SBUF.
  conv4..conv6 additionally run in fp8 DoubleRow mode (2 MACs/cell/cycle):
      activations for a pair of 128-channel groups live in one flat
      [128, 2, S] buffer (padded images stored contiguously incl. halos,
      plus guard columns), so the moving operand is a contiguous run
      [K, 2, N] over full padded images; conv outputs at halo positions are
      garbage and are simply never read by the strided post-processing.
  relu/maxpool: relu and (positive) BN scale commute with max, so pool after
      the fused (max(x,0)*scale) tensor_scalar; then ACT Sign (+bias) writes
      the fp8 binarized input of the next layer.
  dense+softmax: fp32 matmuls (h6 as stationary operand), bias via a K=1
      matmul against a ones vector, softmax with exp/accum on ACT.

All element-wise fp32 ops follow the reference's rounding sequence
(mul-round-add-round), so layers 2..6 are bit-exact vs the JAX reference;
the only inexactness is conv1 accumulation order and dense/softmax rounding.
"""

import numpy as np
import ml_dtypes

_F8 = ml_dtypes.float8_e4m3

B = 16        # images per core
N_CORES = 8

# (layer, Gi, Go, H, W, pool, doublerow)
_LAYERS = [
    (2, 1, 1, 32, 32, True, False),
    (3, 1, 2, 16, 16, False, False),
    (4, 2, 2, 16, 16, True, True),
    (5, 2, 4, 8, 8, False, True),
    (6, 4, 4, 8, 8, True, True),
]
_WCOLS = {2: 1152, 3: 2304, 4: 4608, 5: 9216, 6: 18432}


def _flat_geom(H, W):
    """Geometry of the flat padded pair-buffers for DoubleRow layers."""
    hp, wp = H + 2, W + 2
    img = hp * wp
    g = ((wp + 1 + 15) // 16) * 16      # guard >= wp+1, multiple of 16
    s = B * img + 2 * g
    assert s % 16 == 0
    return hp, wp, img, g, s


_prog_cache = {}


def _build_program():
    """Build + compile the SPMD Bass program (once per process)."""
    if "nc" in _prog_cache:
        return _prog_cache["nc"]

    from contextlib import ExitStack

    import concourse.bacc as bacc
    import concourse.mybir as mybir
    import concourse.tile as tile

    dt = mybir.dt
    AL = mybir.AluOpType
    AF = mybir.ActivationFunctionType
    AX = mybir.AxisListType
    DR = mybir.MatmulPerfMode.DoubleRow

    nc = bacc.Bacc("TRN2", target_bir_lowering=False, debug=False,
                   num_devices=N_CORES)

    f32 = dt.float32
    f8 = dt.float8e4

    d_xcol = nc.dram_tensor("xcol", [128, 4096], f32, kind="ExternalInput").ap()
    d_w1p = nc.dram_tensor("w1p", [128, 128], f32, kind="ExternalInput").ap()
    d_bnv = nc.dram_tensor("bnv", [128, 29], f32, kind="ExternalInput").ap()
    d_dwp = nc.dram_tensor("dwp", [128, 40], f32, kind="ExternalInput").ap()
    d_db = nc.dram_tensor("db", [1, 10], f32, kind="ExternalInput").ap()
    d_w = {l: nc.dram_tensor(f"wb{l}", [128, _WCOLS[l]], f8,
                             kind="ExternalInput").ap()
           for l, *_ in _LAYERS}
    d_out = nc.dram_tensor("out", [256, 10], f32, kind="ExternalOutput").ap()

    g4 = _flat_geom(16, 16)   # a4 geometry (L4 input 16x16)
    g5 = _flat_geom(8, 8)     # a5
    g6 = _flat_geom(8, 8)     # a6

    with tile.TileContext(nc) as tc, ExitStack() as ctx:
        consts = ctx.enter_context(tc.tile_pool(name="consts", bufs=1))
        psum_pool = ctx.enter_context(
            tc.tile_pool(name="cpsum", bufs=6, space="PSUM"))
        psum_d = ctx.enter_context(
            tc.tile_pool(name="dpsum", bufs=2, space="PSUM"))
        tmps = ctx.enter_context(tc.tile_pool(name="tmps", bufs=4))
        small = ctx.enter_context(tc.tile_pool(name="small", bufs=2))

        # ---- constant loads -------------------------------------------------
        xcol_sb = consts.tile([128, 4096], f32, tag="xcol")
        nc.sync.dma_start(xcol_sb[:], d_xcol)
        w1_sb = consts.tile([128, 128], f32, tag="w1p")
        nc.sync.dma_start(w1_sb[:], d_w1p)
        bn_sb = consts.tile([128, 29], f32, tag="bnv")
        nc.sync.dma_start(bn_sb[:], d_bnv)
        dwp_sb = consts.tile([128, 40], f32, tag="dwp")
        nc.sync.dma_start(dwp_sb[:], d_dwp)
        db_sb = consts.tile([1, 10], f32, tag="db")
        nc.sync.dma_start(db_sb[:], d_db)
        w_sb = {}
        for l, *_ in _LAYERS:
            w_sb[l] = consts.tile([128, _WCOLS[l]], f8, tag=f"wb{l}",
                                  name=f"wb{l}")
            nc.sync.dma_start(w_sb[l][:], d_w[l])
        ones_sb = consts.tile([1, 128], f32, tag="ones")
        nc.vector.memset(ones_sb[:], 1.0)

        # ---- activation buffers --------------------------------------------
        a2 = consts.tile([128, B, 34, 34], f8, tag="a2", name="a2")
        nc.gpsimd.memset(a2[:], 0.0)
        a3 = consts.tile([128, B, 18, 18], f8, tag="a3", name="a3")
        nc.gpsimd.memset(a3[:], 0.0)
        a4 = consts.tile([128, 2, g4[4]], f8, tag="a4", name="a4")
        nc.gpsimd.memset(a4[:], 0.0)
        a5 = consts.tile([128, 2, g5[4]], f8, tag="a5", name="a5")
        nc.gpsimd.memset(a5[:], 0.0)
        a6 = [consts.tile([128, 2, g6[4]], f8, tag=f"a6{i}", name=f"a6{i}")
              for i in range(2)]
        for t in a6:
            nc.gpsimd.memset(t[:], 0.0)
        h6 = [consts.tile([128, B, 4, 4], f32, tag=f"h6{i}", name=f"h6{i}")
              for i in range(4)]

        def flat_view(t, geom):
            """[128,2,S] -> [128, 2, B, Hp, Wp] view of the guarded region."""
            hp, wp, img, g, s = geom
            return t[:, :, g:g + B * img].rearrange(
                "p j (b h w) -> p j b h w", b=B, h=hp, w=wp)

        a4v = flat_view(a4, g4)
        a5v = flat_view(a5, g5)
        a6v = [flat_view(t, g6) for t in a6]

        bn_cols = {1: (1, 2), 2: (3, 4), 3: (5, 7), 4: (9, 11),
                   5: (13, 17), 6: (21, 25)}
        b1_ap = bn_sb[:, 0:1]

        # ---- layer 1: fp32 conv via 4-way row-tiled K=27 matmuls ------------
        s1_ap = bn_sb[:, 1:2]
        bb1_ap = bn_sb[:, 2:3]
        for ns in range(8):
            for g in range(4):      # row-group inner: adjacent MMs hit
                b = 4 * g + ns // 2  # different 32-row PE quadrants -> overlap
                y0 = (ns % 2) * 16
                pt = psum_pool.tile([128, 16, 32], f32, tag="cps", name="cps")
                lhsT = w1_sb[32 * g:32 * g + 27, :]
                rhs = xcol_sb[32 * g:32 * g + 27, 512 * ns:512 * (ns + 1)]
                nc.tensor.matmul(pt[:, :, :], lhsT, rhs, start=True, stop=True,
                                 tile_position=(32 * g, 0))
                r = tmps.tile([128, 16, 32], f32, tag="rl1", name="rl1")
                nc.vector.tensor_scalar(r[:], pt[:, :, :], b1_ap, 0.0,
                                        AL.add, AL.max)
                dest = a2[:, b, 1 + y0:17 + y0, 1:33]
                nc.scalar.activation(dest, r[:], AF.Sign,
                                     bias=bb1_ap, scale=s1_ap)

        def write_next(l, go, b0, nb, src_ap, H2, W2, y0=0):
            """Write binarized/affine output into layer l+1's input buffer."""
            b_ap = bn_sb[:, bn_cols[l][1] + go:bn_cols[l][1] + go + 1]
            if l == 2:
                dest = a3[:, b0, 1 + y0 // 2:1 + y0 // 2 + H2, 1:1 + W2]
                nc.scalar.activation(dest, src_ap, AF.Sign, bias=b_ap)
            elif l == 3:
                dest = a4v[:, go, b0:b0 + nb, 1:1 + H2, 1:1 + W2]
                nc.scalar.activation(dest, src_ap, AF.Sign, bias=b_ap)
            elif l == 4:
                dest = a5v[:, go, b0:b0 + nb, 1:1 + H2, 1:1 + W2]
                nc.scalar.activation(dest, src_ap, AF.Sign, bias=b_ap)
            elif l == 5:
                dest = a6v[go // 2][:, go % 2, b0:b0 + nb, 1:1 + H2, 1:1 + W2]
                nc.scalar.activation(dest, src_ap, AF.Sign, bias=b_ap)
            else:
                dest = h6[go][:, b0:b0 + nb, :, :]
                nc.scalar.activation(dest, src_ap, AF.Identity, bias=b_ap)

        # ---- binary conv layers L2/L3 (classic shifted-AP path) -------------
        for (l, Gi, Go, H, W, pool, dr) in _LAYERS:
            if dr:
                continue
            sc0, _ = bn_cols[l]
            acts = {2: [a2], 3: [a3]}[l]
            if H * W >= 512:
                nb, rows = 1, 512 // W
                chunks_per_img = H // rows
                nchunks = B * chunks_per_img
            else:
                nb = 512 // (H * W)
                rows = nb * H
                nchunks = B // nb
            for go in range(Go):
                s_ap = bn_sb[:, sc0 + go:sc0 + go + 1]
                for c in range(nchunks):
                    if nb == 1:
                        b0, y0 = c // chunks_per_img, (c % chunks_per_img) * rows
                    else:
                        b0, y0 = c * nb, 0
                    pt = psum_pool.tile([128, rows, W], f32, tag="cps",
                                        name="cps")
                    i_mm, n_mm = 0, Gi * 9
                    for gi in range(Gi):
                        for k in range(9):
                            dy, dx = k // 3, k % 3
                            col = ((gi * Go + go) * 9 + k) * 128
                            lhsT = w_sb[l][:, col:col + 128]
                            if nb == 1:
                                rhs = acts[gi][:, b0, y0 + dy:y0 + dy + rows,
                                               dx:dx + W]
                            else:
                                rhs = acts[gi][:, b0:b0 + nb, dy:dy + H,
                                               dx:dx + W]
                            nc.tensor.matmul(pt[:, :, :], lhsT, rhs,
                                             start=(i_mm == 0),
                                             stop=(i_mm == n_mm - 1))
                            i_mm += 1
                    # relu+scale from PSUM; s>0 commutes with max
                    ts = tmps.tile([128, rows, W], f32, tag="ts", name="ts")
                    nc.vector.tensor_scalar(ts[:], pt[:, :, :], 0.0, s_ap,
                                            AL.max, AL.mult)
                    if pool:
                        vx = ts[:].rearrange("p r (a two) -> p r a two", two=2)
                        tx = tmps.tile([128, rows, W // 2], f32, tag="tx",
                                       name="tx")
                        nc.vector.tensor_tensor(tx[:], vx[:, :, :, 0],
                                                vx[:, :, :, 1], op=AL.max)
                        vy = tx[:].rearrange("p (a two) x -> p a two x", two=2)
                        tp = tmps.tile([128, rows // 2, W // 2], f32, tag="tp",
                                       name="tp")
                        nc.vector.tensor_tensor(tp[:], vy[:, :, 0, :],
                                                vy[:, :, 1, :], op=AL.max)
                        if nb == 1:
                            write_next(l, go, b0, 1, tp[:], rows // 2, W // 2,
                                       y0=y0)
                        else:
                            write_next(l, go, b0, nb,
                                       tp[:].rearrange("p (b y) x -> p b y x",
                                                       b=nb),
                                       H // 2, W // 2)
                    else:
                        write_next(l, go, b0, nb,
                                   ts[:].rearrange("p (b y) x -> p b y x",
                                                   b=nb), H, W)

        # ---- binary conv layers L4/L5/L6 (fp8 DoubleRow, flat runs) ---------
        for (l, Gi, Go, H, W, pool, dr) in _LAYERS:
            if not dr:
                continue
            sc0, _ = bn_cols[l]
            geom = {4: g4, 5: g5, 6: g6}[l]
            hp, wp, img, gd, s = geom
            srcs = {4: [a4], 5: [a5], 6: a6}[l]
            npairs = Gi // 2
            nb = 1                            # images per chunk (divisor of B)
            while nb * 2 <= B and nb * 2 * img <= 512:
                nb *= 2
            N = nb * img
            nchunks = B // nb
            for go in range(Go):
                s_ap = bn_sb[:, sc0 + go:sc0 + go + 1]
                for c in range(nchunks):
                    b0 = c * nb
                    pt = psum_pool.tile([128, N], f32, tag="cps", name="cps")
                    i_mm, n_mm = 0, npairs * 9
                    for pr in range(npairs):
                        for k in range(9):
                            dy, dx = k // 3, k % 3
                            base = ((pr * Go + go) * 9 + k) * 256
                            lhsT = w_sb[l][:, base:base + 256].rearrange(
                                "p (j c) -> p j c", j=2)
                            off = gd + b0 * img + (dy - 1) * wp + (dx - 1)
                            rhs = srcs[pr][:, :, off:off + N]
                            nc.tensor.matmul(pt[:, :], lhsT, rhs,
                                             start=(i_mm == 0),
                                             stop=(i_mm == n_mm - 1),
                                             perf_mode=DR)
                            i_mm += 1
                    # interior view of the padded-grid conv output
                    ptv = pt[:].rearrange("p (b h w) -> p b h w",
                                          b=nb, h=hp, w=wp)
                    inter = ptv[:, :, 1:1 + H, 1:1 + W]
                    ts = tmps.tile([128, nb, H, W], f32, tag="ts", name="ts")
                    nc.vector.tensor_scalar(ts[:], inter, 0.0, s_ap,
                                            AL.max, AL.mult)
                    if pool:
                        vx = ts[:].rearrange("p b h (x two) -> p b h x two",
                                             two=2)
                        tx = tmps.tile([128, nb, H, W // 2], f32, tag="tx",
                                       name="tx")
                        nc.vector.tensor_tensor(tx[:], vx[:, :, :, :, 0],
                                                vx[:, :, :, :, 1], op=AL.max)
                        vy = tx[:].rearrange("p b (y two) x -> p b y two x",
                                             two=2)
                        tp = tmps.tile([128, nb, H // 2, W // 2], f32,
                                       tag="tp", name="tp")
                        nc.vector.tensor_tensor(tp[:], vy[:, :, :, 0, :],
                                                vy[:, :, :, 1, :], op=AL.max)
                        write_next(l, go, b0, nb, tp[:], H // 2, W // 2)
                    else:
                        write_next(l, go, b0, nb, ts[:], H, W)

        # ---- dense + softmax ------------------------------------------------
        for p in range(2):
            ptd = psum_d.tile([128, 10], f32, tag="dps", name="dps")
            for gi in range(4):
                lhsT = h6[gi][:, 8 * p:8 * p + 8, :, :]
                rhs = dwp_sb[:, gi * 10:(gi + 1) * 10]
                nc.tensor.matmul(ptd[:, :], lhsT, rhs,
                                 start=(gi == 0), stop=False)
            nc.tensor.matmul(ptd[:, :], ones_sb[0:1, :], db_sb[0:1, :],
                             start=False, stop=True)
            mx = small.tile([128, 1], f32, tag="mx", name="mx")
            nc.vector.tensor_reduce(mx[:], ptd[:, :], axis=AX.X, op=AL.max,
                                    negate=True)
            e = small.tile([128, 10], f32, tag="e", name="e")
            ssum = small.tile([128, 1], f32, tag="ssum", name="ssum")
            nc.scalar.activation(e[:], ptd[:, :], AF.Exp, bias=mx[:],
                                 scale=1.0, accum_out=ssum[:])
            rcp = small.tile([128, 1], f32, tag="rcp", name="rcp")
            nc.vector.reciprocal(rcp[:], ssum[:])
            o = small.tile([128, 10], f32, tag="o", name="o")
            nc.vector.tensor_scalar(o[:], e[:], rcp[:], None, AL.mult)
            nc.sync.dma_start(d_out[128 * p:128 * (p + 1), :], o[:])

    nc.compile()
    _prog_cache["nc"] = nc
    return nc


# --------------------------------------------------------------------------
# host-side input packing
# --------------------------------------------------------------------------

def _pack_shared(inputs):
    w1 = np.asarray(inputs["w1"], np.float32)
    w1flat = w1.reshape(27, 128)          # row r = (ky*3+kx)*3 + ci
    w1p = np.zeros((128, 128), np.float32)
    for g in range(4):
        w1p[32 * g:32 * g + 27, :] = w1flat

    bnv = np.zeros((128, 29), np.float32)
    bnv[:, 0] = np.asarray(inputs["b1"], np.float32)
    bnv[:, 1] = np.asarray(inputs["bn1_scale"], np.float32)
    bnv[:, 2] = np.asarray(inputs["bn1_bias"], np.float32)
    bn_cols = {2: (3, 4), 3: (5, 7), 4: (9, 11), 5: (13, 17), 6: (21, 25)}
    for l, (sc, bc) in bn_cols.items():
        s = np.asarray(inputs[f"bn{l}_scale"], np.float32)
        b = np.asarray(inputs[f"bn{l}_bias"], np.float32)
        g = s.size // 128
        bnv[:, sc:sc + g] = s.reshape(g, 128).T
        bnv[:, bc:bc + g] = b.reshape(g, 128).T

    wbs = {}
    for (l, Gi, Go, _, _, _, dr) in _LAYERS:
        w = np.asarray(inputs[f"w{l}"], np.float32)
        ws = np.sign(w).astype(_F8)       # (3,3,Cin,Cout)
        blob = np.empty((128, _WCOLS[l]), _F8)
        if not dr:
            for gi in range(Gi):
                for go in range(Go):
                    for k in range(9):
                        col = ((gi * Go + go) * 9 + k) * 128
                        blob[:, col:col + 128] = ws[k // 3, k % 3,
                                                    gi * 128:(gi + 1) * 128,
                                                    go * 128:(go + 1) * 128]
        else:
            for pr in range(Gi // 2):
                for go in range(Go):
                    for k in range(9):
                        base = ((pr * Go + go) * 9 + k) * 256
                        for j in range(2):
                            ci0 = (2 * pr + j) * 128
                            blob[:, base + j * 128:base + (j + 1) * 128] = \
                                ws[k // 3, k % 3, ci0:ci0 + 128,
                                   go * 128:(go + 1) * 128]
        wbs[l] = blob

    dw = np.asarray(inputs["dense_w"], np.float32)
    dwp = dw.reshape(4, 128, 10).transpose(1, 0, 2).reshape(128, 40).copy()
    db = np.asarray(inputs["dense_b"], np.float32).reshape(1, 10).copy()
    return w1p, bnv, wbs, dwp, db


def _pack_xcol(x16):
    """[16,32,32,3] f32 -> [128,4096] 4-way row-group packed im2col."""
    xp = np.zeros((B, 34, 34, 3), np.float32)
    xp[:, 1:33, 1:33, :] = x16
    cols = np.empty((27, B, 32, 32), np.float32)
    for ky in range(3):
        for kx in range(3):
            for ci in range(3):
                r = (ky * 3 + kx) * 3 + ci
                cols[r] = xp[:, ky:ky + 32, kx:kx + 32, ci]
    cols = cols.reshape(27, B * 1024)
    xcol = np.zeros((128, 4096), np.float32)
    for g in range(4):
        xcol[32 * g:32 * g + 27, :] = cols[:, 4096 * g:4096 * (g + 1)]
    return xcol


def _make_in_maps(inputs):
    w1p, bnv, wbs, dwp, db = _pack_shared(inputs)
    x = np.asarray(inputs["x"], np.float32)
    in_maps = []
    for c in range(N_CORES):
        m = {"xcol": _pack_xcol(x[B * c:B * (c + 1)]),
             "w1p": w1p, "bnv": bnv, "dwp": dwp, "db": db}
        for l in wbs:
            m[f"wb{l}"] = wbs[l]
        in_maps.append(m)
    return in_maps


def _run(inputs, trace=False):
    """Returns (output [128,4,4,10] f32, BassKernelResults)."""
    nc = _build_program()
    from concourse.bass_utils import run_bass_kernel_spmd
    in_maps = _make_in_maps(inputs)
    res = run_bass_kernel_spmd(nc, in_maps, list(range(N_CORES)), trace=trace)
    outs = [res.results[c]["out"].reshape(B, 4, 4, 10)
            for c in range(N_CORES)]
    return np.concatenate(outs, axis=0), res


def kernel(**inputs):
    out, _ = _run(inputs)
    return out


# revision 7
# speedup vs baseline: 1.3048x; 1.0329x over previous
"""Trainium2 Bass kernel for a 6-layer binary CNN (XNOR-net style).

Contract: kernel(**inputs) takes the FULL unsharded inputs (batch 128) and
returns the FULL output [128, 4, 4, 10] float32.

Strategy
--------
Pure data parallel: batch 128 -> 16 images on each of 8 NeuronCores; all
weights replicated. One SPMD Bass program, per-core input maps.

Per core:
  conv1 (3->128, fp32):  host-side im2col (K=27) packed 4-way into partition
      row-groups; 4-way row-tiled fp32 matmuls on the PE.
  conv2..conv6 (binary): sign(w) and sign(act) are exactly representable in
      fp8e4m3; products are +-1 and PSUM accumulates in fp32 -> the conv sums
      are EXACT integers.  3x3 SAME conv = 9 shifted matmuls accumulating in
      PSUM, reading from zero-haloed padded activation buffers in SBUF.
  conv4..conv6 additionally run in fp8 DoubleRow mode (2 MACs/cell/cycle):
      activations for a pair of 128-channel groups live in one flat
      [128, 2, S] buffer (padded images stored contiguously incl. halos,
      plus guard columns), so the moving operand is a contiguous run
      [K, 2, N] over full padded images; conv outputs at halo positions are
      garbage and are simply never read by the strided post-processing.
  relu/maxpool: relu and (positive) BN scale commute with max, so pool after
      the fused (max(x,0)*scale) tensor_scalar; then ACT Sign (+bias) writes
      the fp8 binarized input of the next layer.
  dense+softmax: fp32 matmuls (h6 as stationary operand), bias via a K=1
      matmul against a ones vector, softmax with exp/accum on ACT.

All element-wise fp32 ops follow the reference's rounding sequence
(mul-round-add-round), so layers 2..6 are bit-exact vs the JAX reference;
the only inexactness is conv1 accumulation order and dense/softmax rounding.
"""

import numpy as np
import ml_dtypes

_F8 = ml_dtypes.float8_e4m3

B = 16        # images per core
N_CORES = 8

# (layer, Gi, Go, H, W, pool, doublerow)
_LAYERS = [
    (2, 1, 1, 32, 32, True, False),
    (3, 1, 2, 16, 16, False, False),
    (4, 2, 2, 16, 16, True, True),
    (5, 2, 4, 8, 8, False, True),
    (6, 4, 4, 8, 8, True, True),
]
_WCOLS = {2: 1152, 3: 2304, 4: 4608, 5: 9216, 6: 18432}


def _flat_geom(H, W):
    """Geometry of the flat padded pair-buffers for DoubleRow layers."""
    hp, wp = H + 2, W + 2
    img = hp * wp
    g = ((wp + 1 + 15) // 16) * 16      # guard >= wp+1, multiple of 16
    s = B * img + 2 * g
    assert s % 16 == 0
    return hp, wp, img, g, s


_prog_cache = {}


def _build_program():
    """Build + compile the SPMD Bass program (once per process)."""
    if "nc" in _prog_cache:
        return _prog_cache["nc"]

    from contextlib import ExitStack

    import concourse.bacc as bacc
    import concourse.mybir as mybir
    import concourse.tile as tile

    dt = mybir.dt
    AL = mybir.AluOpType
    AF = mybir.ActivationFunctionType
    AX = mybir.AxisListType
    DR = mybir.MatmulPerfMode.DoubleRow

    nc = bacc.Bacc("TRN2", target_bir_lowering=False, debug=False,
                   num_devices=N_CORES)

    f32 = dt.float32
    f8 = dt.float8e4

    d_xcol = nc.dram_tensor("xcol", [128, 4096], f32, kind="ExternalInput").ap()
    d_w1p = nc.dram_tensor("w1p", [128, 128], f32, kind="ExternalInput").ap()
    d_bnv = nc.dram_tensor("bnv", [128, 29], f32, kind="ExternalInput").ap()
    d_dwp = nc.dram_tensor("dwp", [128, 40], f32, kind="ExternalInput").ap()
    d_db = nc.dram_tensor("db", [1, 10], f32, kind="ExternalInput").ap()
    d_w = {l: nc.dram_tensor(f"wb{l}", [128, _WCOLS[l]], f8,
                             kind="ExternalInput").ap()
           for l, *_ in _LAYERS}
    d_out = nc.dram_tensor("out", [256, 10], f32, kind="ExternalOutput").ap()

    g4 = _flat_geom(16, 16)   # a4 geometry (L4 input 16x16)
    g5 = _flat_geom(8, 8)     # a5
    g6 = _flat_geom(8, 8)     # a6

    with tile.TileContext(nc) as tc, ExitStack() as ctx:
        consts = ctx.enter_context(tc.tile_pool(name="consts", bufs=1))
        psum_pool = ctx.enter_context(
            tc.tile_pool(name="cpsum", bufs=6, space="PSUM"))
        psum_d = ctx.enter_context(
            tc.tile_pool(name="dpsum", bufs=2, space="PSUM"))
        tmps = ctx.enter_context(tc.tile_pool(name="tmps", bufs=4))
        small = ctx.enter_context(tc.tile_pool(name="small", bufs=2))

        # ---- constant loads -------------------------------------------------
        xcol_sb = consts.tile([128, 4096], f32, tag="xcol")
        nc.sync.dma_start(xcol_sb[:], d_xcol)
        w1_sb = consts.tile([128, 128], f32, tag="w1p")
        nc.sync.dma_start(w1_sb[:], d_w1p)
        bn_sb = consts.tile([128, 29], f32, tag="bnv")
        nc.sync.dma_start(bn_sb[:], d_bnv)
        dwp_sb = consts.tile([128, 40], f32, tag="dwp")
        nc.sync.dma_start(dwp_sb[:], d_dwp)
        db_sb = consts.tile([1, 10], f32, tag="db")
        nc.sync.dma_start(db_sb[:], d_db)
        w_sb = {}
        for l, *_ in _LAYERS:
            w_sb[l] = consts.tile([128, _WCOLS[l]], f8, tag=f"wb{l}",
                                  name=f"wb{l}")
            nc.sync.dma_start(w_sb[l][:], d_w[l])
        ones_sb = consts.tile([1, 128], f32, tag="ones")
        nc.vector.memset(ones_sb[:], 1.0)

        # ---- activation buffers --------------------------------------------
        a2 = consts.tile([128, B, 34, 34], f8, tag="a2", name="a2")
        nc.gpsimd.memset(a2[:], 0.0)
        a3 = consts.tile([128, B, 18, 18], f8, tag="a3", name="a3")
        nc.gpsimd.memset(a3[:], 0.0)
        a4 = consts.tile([128, 2, g4[4]], f8, tag="a4", name="a4")
        nc.gpsimd.memset(a4[:], 0.0)
        a5 = consts.tile([128, 2, g5[4]], f8, tag="a5", name="a5")
        nc.gpsimd.memset(a5[:], 0.0)
        a6 = [consts.tile([128, 2, g6[4]], f8, tag=f"a6{i}", name=f"a6{i}")
              for i in range(2)]
        for t in a6:
            nc.gpsimd.memset(t[:], 0.0)
        h6 = [consts.tile([128, B, 4, 4], f32, tag=f"h6{i}", name=f"h6{i}")
              for i in range(4)]

        def flat_view(t, geom):
            """[128,2,S] -> [128, 2, B, Hp, Wp] view of the guarded region."""
            hp, wp, img, g, s = geom
            return t[:, :, g:g + B * img].rearrange(
                "p j (b h w) -> p j b h w", b=B, h=hp, w=wp)

        a4v = flat_view(a4, g4)
        a5v = flat_view(a5, g5)
        a6v = [flat_view(t, g6) for t in a6]

        bn_cols = {1: (1, 2), 2: (3, 4), 3: (5, 7), 4: (9, 11),
                   5: (13, 17), 6: (21, 25)}
        b1_ap = bn_sb[:, 0:1]

        # ---- layer 1: fp32 conv via 4-way row-tiled K=27 matmuls ------------
        s1_ap = bn_sb[:, 1:2]
        bb1_ap = bn_sb[:, 2:3]
        for ns in range(8):
            for g in range(4):      # row-group inner: adjacent MMs hit
                b = 4 * g + ns // 2  # different 32-row PE quadrants -> overlap
                y0 = (ns % 2) * 16
                pt = psum_pool.tile([128, 16, 32], f32, tag="cps", name="cps")
                lhsT = w1_sb[32 * g:32 * g + 27, :]
                rhs = xcol_sb[32 * g:32 * g + 27, 512 * ns:512 * (ns + 1)]
                nc.tensor.matmul(pt[:, :, :], lhsT, rhs, start=True, stop=True,
                                 tile_position=(32 * g, 0))
                r = tmps.tile([128, 16, 32], f32, tag="rl1", name="rl1")
                nc.vector.tensor_scalar(r[:], pt[:, :, :], b1_ap, 0.0,
                                        AL.add, AL.max)
                dest = a2[:, b, 1 + y0:17 + y0, 1:33]
                nc.scalar.activation(dest, r[:], AF.Sign,
                                     bias=bb1_ap, scale=s1_ap)

        def write_next(l, go, b0, nb, src_ap, H2, W2, y0=0):
            """Write binarized/affine output into layer l+1's input buffer."""
            b_ap = bn_sb[:, bn_cols[l][1] + go:bn_cols[l][1] + go + 1]
            if l == 2:
                dest = a3[:, b0, 1 + y0 // 2:1 + y0 // 2 + H2, 1:1 + W2]
                nc.scalar.activation(dest, src_ap, AF.Sign, bias=b_ap)
            elif l == 3:
                dest = a4v[:, go, b0:b0 + nb, 1:1 + H2, 1:1 + W2]
                nc.scalar.activation(dest, src_ap, AF.Sign, bias=b_ap)
            elif l == 4:
                dest = a5v[:, go, b0:b0 + nb, 1:1 + H2, 1:1 + W2]
                nc.scalar.activation(dest, src_ap, AF.Sign, bias=b_ap)
            elif l == 5:
                dest = a6v[go // 2][:, go % 2, b0:b0 + nb, 1:1 + H2, 1:1 + W2]
                nc.scalar.activation(dest, src_ap, AF.Sign, bias=b_ap)
            else:
                dest = h6[go][:, b0:b0 + nb, :, :]
                nc.scalar.activation(dest, src_ap, AF.Identity, bias=b_ap)

        # ---- binary conv layers L2/L3 (classic shifted-AP path) -------------
        for (l, Gi, Go, H, W, pool, dr) in _LAYERS:
            if dr:
                continue
            sc0, _ = bn_cols[l]
            acts = {2: [a2], 3: [a3]}[l]
            if H * W >= 512:
                nb, rows = 1, 512 // W
                chunks_per_img = H // rows
                nchunks = B * chunks_per_img
            else:
                nb = 512 // (H * W)
                rows = nb * H
                nchunks = B // nb
            for go in range(Go):
                s_ap = bn_sb[:, sc0 + go:sc0 + go + 1]
                for c in range(nchunks):
                    if nb == 1:
                        b0, y0 = c // chunks_per_img, (c % chunks_per_img) * rows
                    else:
                        b0, y0 = c * nb, 0
                    pt = psum_pool.tile([128, rows, W], f32, tag="cps",
                                        name="cps")
                    i_mm, n_mm = 0, Gi * 9
                    for gi in range(Gi):
                        for k in range(9):
                            dy, dx = k // 3, k % 3
                            col = ((gi * Go + go) * 9 + k) * 128
                            lhsT = w_sb[l][:, col:col + 128]
                            if nb == 1:
                                rhs = acts[gi][:, b0, y0 + dy:y0 + dy + rows,
                                               dx:dx + W]
                            else:
                                rhs = acts[gi][:, b0:b0 + nb, dy:dy + H,
                                               dx:dx + W]
                            nc.tensor.matmul(pt[:, :, :], lhsT, rhs,
                                             start=(i_mm == 0),
                                             stop=(i_mm == n_mm - 1))
                            i_mm += 1
                    # relu+scale from PSUM; s>0 commutes with max
                    ts = tmps.tile([128, rows, W], f32, tag="ts", name="ts")
                    nc.vector.tensor_scalar(ts[:], pt[:, :, :], 0.0, s_ap,
                                            AL.max, AL.mult)
                    if pool:
                        vx = ts[:].rearrange("p r (a two) -> p r a two", two=2)
                        tx = tmps.tile([128, rows, W // 2], f32, tag="tx",
                                       name="tx")
                        nc.vector.tensor_tensor(tx[:], vx[:, :, :, 0],
                                                vx[:, :, :, 1], op=AL.max)
                        vy = tx[:].rearrange("p (a two) x -> p a two x", two=2)
                        tp = tmps.tile([128, rows // 2, W // 2], f32, tag="tp",
                                       name="tp")
                        nc.vector.tensor_tensor(tp[:], vy[:, :, 0, :],
                                                vy[:, :, 1, :], op=AL.max)
                        if nb == 1:
                            write_next(l, go, b0, 1, tp[:], rows // 2, W // 2,
                                       y0=y0)
                        else:
                            write_next(l, go, b0, nb,
                                       tp[:].rearrange("p (b y) x -> p b y x",
                                                       b=nb),
                                       H // 2, W // 2)
                    else:
                        write_next(l, go, b0, nb,
                                   ts[:].rearrange("p (b y) x -> p b y x",
                                                   b=nb), H, W)

        # ---- binary conv layers L4/L5/L6 (fp8 DoubleRow, flat runs) ---------
        for (l, Gi, Go, H, W, pool, dr) in _LAYERS:
            if not dr:
                continue
            sc0, _ = bn_cols[l]
            geom = {4: g4, 5: g5, 6: g6}[l]
            hp, wp, img, gd, s = geom
            srcs = {4: [a4], 5: [a5], 6: a6}[l]
            npairs = Gi // 2
            nb = 1                            # images per chunk (divisor of B)
            while nb * 2 <= B and nb * 2 * img - 2 * wp <= 512:
                nb *= 2
            N = nb * img - 2 * wp            # trim top/bottom halo rows
            nchunks = B // nb
            for go in range(Go):
                s_ap = bn_sb[:, sc0 + go:sc0 + go + 1]
                for c in range(nchunks):
                    b0 = c * nb
                    pt = psum_pool.tile([128, nb * img], f32, tag="cps",
                                        name="cps")
                    i_mm, n_mm = 0, npairs * 9
                    for pr in range(npairs):
                        for k in range(9):
                            dy, dx = k // 3, k % 3
                            base = ((pr * Go + go) * 9 + k) * 256
                            lhsT = w_sb[l][:, base:base + 256].rearrange(
                                "p (j c) -> p j c", j=2)
                            off = gd + b0 * img + wp + (dy - 1) * wp + (dx - 1)
                            rhs = srcs[pr][:, :, off:off + N]
                            nc.tensor.matmul(pt[:, wp:wp + N], lhsT, rhs,
                                             start=(i_mm == 0),
                                             stop=(i_mm == n_mm - 1),
                                             perf_mode=DR)
                            i_mm += 1
                    # interior view of the padded-grid conv output
                    ptv = pt[:].rearrange("p (b h w) -> p b h w",
                                          b=nb, h=hp, w=wp)
                    inter = ptv[:, :, 1:1 + H, 1:1 + W]
                    ts = tmps.tile([128, nb, H, W], f32, tag="ts", name="ts")
                    nc.vector.tensor_scalar(ts[:], inter, 0.0, s_ap,
                                            AL.max, AL.mult)
                    if pool:
                        vx = ts[:].rearrange("p b h (x two) -> p b h x two",
                                             two=2)
                        tx = tmps.tile([128, nb, H, W // 2], f32, tag="tx",
                                       name="tx")
                        nc.vector.tensor_tensor(tx[:], vx[:, :, :, :, 0],
                                                vx[:, :, :, :, 1], op=AL.max)
                        vy = tx[:].rearrange("p b (y two) x -> p b y two x",
                                             two=2)
                        tp = tmps.tile([128, nb, H // 2, W // 2], f32,
                                       tag="tp", name="tp")
                        nc.vector.tensor_tensor(tp[:], vy[:, :, :, 0, :],
                                                vy[:, :, :, 1, :], op=AL.max)
                        write_next(l, go, b0, nb, tp[:], H // 2, W // 2)
                    else:
                        write_next(l, go, b0, nb, ts[:], H, W)

        # ---- dense + softmax ------------------------------------------------
        for p in range(2):
            ptd = psum_d.tile([128, 10], f32, tag="dps", name="dps")
            for gi in range(4):
                lhsT = h6[gi][:, 8 * p:8 * p + 8, :, :]
                rhs = dwp_sb[:, gi * 10:(gi + 1) * 10]
                nc.tensor.matmul(ptd[:, :], lhsT, rhs,
                                 start=(gi == 0), stop=False)
            nc.tensor.matmul(ptd[:, :], ones_sb[0:1, :], db_sb[0:1, :],
                             start=False, stop=True)
            mx = small.tile([128, 1], f32, tag="mx", name="mx")
            nc.vector.tensor_reduce(mx[:], ptd[:, :], axis=AX.X, op=AL.max,
                                    negate=True)
            e = small.tile([128, 10], f32, tag="e", name="e")
            ssum = small.tile([128, 1], f32, tag="ssum", name="ssum")
            nc.scalar.activation(e[:], ptd[:, :], AF.Exp, bias=mx[:],
                                 scale=1.0, accum_out=ssum[:])
            rcp = small.tile([128, 1], f32, tag="rcp", name="rcp")
            nc.vector.reciprocal(rcp[:], ssum[:])
            o = small.tile([128, 10], f32, tag="o", name="o")
            nc.vector.tensor_scalar(o[:], e[:], rcp[:], None, AL.mult)
            nc.sync.dma_start(d_out[128 * p:128 * (p + 1), :], o[:])

    nc.compile()
    _prog_cache["nc"] = nc
    return nc


# --------------------------------------------------------------------------
# host-side input packing
# --------------------------------------------------------------------------

def _pack_shared(inputs):
    w1 = np.asarray(inputs["w1"], np.float32)
    w1flat = w1.reshape(27, 128)          # row r = (ky*3+kx)*3 + ci
    w1p = np.zeros((128, 128), np.float32)
    for g in range(4):
        w1p[32 * g:32 * g + 27, :] = w1flat

    bnv = np.zeros((128, 29), np.float32)
    bnv[:, 0] = np.asarray(inputs["b1"], np.float32)
    bnv[:, 1] = np.asarray(inputs["bn1_scale"], np.float32)
    bnv[:, 2] = np.asarray(inputs["bn1_bias"], np.float32)
    bn_cols = {2: (3, 4), 3: (5, 7), 4: (9, 11), 5: (13, 17), 6: (21, 25)}
    for l, (sc, bc) in bn_cols.items():
        s = np.asarray(inputs[f"bn{l}_scale"], np.float32)
        b = np.asarray(inputs[f"bn{l}_bias"], np.float32)
        g = s.size // 128
        bnv[:, sc:sc + g] = s.reshape(g, 128).T
        bnv[:, bc:bc + g] = b.reshape(g, 128).T

    wbs = {}
    for (l, Gi, Go, _, _, _, dr) in _LAYERS:
        w = np.asarray(inputs[f"w{l}"], np.float32)
        ws = np.sign(w).astype(_F8)       # (3,3,Cin,Cout)
        blob = np.empty((128, _WCOLS[l]), _F8)
        if not dr:
            for gi in range(Gi):
                for go in range(Go):
                    for k in range(9):
                        col = ((gi * Go + go) * 9 + k) * 128
                        blob[:, col:col + 128] = ws[k // 3, k % 3,
                                                    gi * 128:(gi + 1) * 128,
                                                    go * 128:(go + 1) * 128]
        else:
            for pr in range(Gi // 2):
                for go in range(Go):
                    for k in range(9):
                        base = ((pr * Go + go) * 9 + k) * 256
                        for j in range(2):
                            ci0 = (2 * pr + j) * 128
                            blob[:, base + j * 128:base + (j + 1) * 128] = \
                                ws[k // 3, k % 3, ci0:ci0 + 128,
                                   go * 128:(go + 1) * 128]
        wbs[l] = blob

    dw = np.asarray(inputs["dense_w"], np.float32)
    dwp = dw.reshape(4, 128, 10).transpose(1, 0, 2).reshape(128, 40).copy()
    db = np.asarray(inputs["dense_b"], np.float32).reshape(1, 10).copy()
    return w1p, bnv, wbs, dwp, db


def _pack_xcol(x16):
    """[16,32,32,3] f32 -> [128,4096] 4-way row-group packed im2col."""
    xp = np.zeros((B, 34, 34, 3), np.float32)
    xp[:, 1:33, 1:33, :] = x16
    cols = np.empty((27, B, 32, 32), np.float32)
    for ky in range(3):
        for kx in range(3):
            for ci in range(3):
                r = (ky * 3 + kx) * 3 + ci
                cols[r] = xp[:, ky:ky + 32, kx:kx + 32, ci]
    cols = cols.reshape(27, B * 1024)
    xcol = np.zeros((128, 4096), np.float32)
    for g in range(4):
        xcol[32 * g:32 * g + 27, :] = cols[:, 4096 * g:4096 * (g + 1)]
    return xcol


def _make_in_maps(inputs):
    w1p, bnv, wbs, dwp, db = _pack_shared(inputs)
    x = np.asarray(inputs["x"], np.float32)
    in_maps = []
    for c in range(N_CORES):
        m = {"xcol": _pack_xcol(x[B * c:B * (c + 1)]),
             "w1p": w1p, "bnv": bnv, "dwp": dwp, "db": db}
        for l in wbs:
            m[f"wb{l}"] = wbs[l]
        in_maps.append(m)
    return in_maps


def _run(inputs, trace=False):
    """Returns (output [128,4,4,10] f32, BassKernelResults)."""
    nc = _build_program()
    from concourse.bass_utils import run_bass_kernel_spmd
    in_maps = _make_in_maps(inputs)
    res = run_bass_kernel_spmd(nc, in_maps, list(range(N_CORES)), trace=trace)
    outs = [res.results[c]["out"].reshape(B, 4, 4, 10)
            for c in range(N_CORES)]
    return np.concatenate(outs, axis=0), res


def kernel(**inputs):
    out, _ = _run(inputs)
    return out
